# revision 23
# baseline (speedup 1.0000x reference)
"""Distributed Bass kernel for sliding-window GQA attention on 8 TRN2 NeuronCores.

Problem: B=2, S=2048, DIM=2048, H=16, KVH=4, HD=128, WINDOW=1024 (causal
sliding window), nonstandard RoPE producing 1.5*HD score features.

Sharding (tensor-parallel on the kv-head axis, data-parallel on batch —
no collectives): core c owns (batch, kv-group) = (c//4, c%4): its 4 q-heads
and 1 kv head over the full 2048-row sequence. wq/wk/wv are column-sharded
by kv group, wo row-sharded. Each core emits a PARTIAL output projection
(its 4 heads x its wo rows); the host sums the 4 partials per batch while
unsharding — replacing the all-reduce.

v2 layout: scores are computed TRANSPOSED (S^T[k, q], K-block stationary and
shared across the 4 heads; the 64-wide rope-imag contraction is row-tiled so
head pairs run concurrently in the PE array). exp goes straight to SBUF as
P^T — no per-block transpose matmuls. Causal/window triangles are applied as
0/1 multiplies on the otherwise-idle GpSimd engine after exp. Row sums come
from a ones-stationary matmul (output replicated across partitions, packed
(h, q)); PV packs all 4 heads into N=512 matmuls against the shared V; the
1/rowsum normalization is fused into the PSUM->SBUF evacuation of PV.
"""
import numpy as np
import ml_dtypes

import concourse.tile as tile
from concourse import bacc, mybir
from concourse.bass_utils import run_bass_kernel_spmd
from contextlib import ExitStack

F32 = mybir.dt.float32
BF16 = mybir.dt.bfloat16
EXP = mybir.ActivationFunctionType.Exp

B, S, DIM = 2, 2048, 2048
H, KVH, HD = 16, 4, 128
HPC = H // KVH  # heads per core (4)
WINDOW = 1024
SCALE = HD ** -0.5
NDC = DIM // 128  # 16 dim chunks
NQC = S // 128    # 16 q/k blocks

_cache = {}


def _build():
    nc = bacc.Bacc("TRN2", target_bir_lowering=False, debug=False, num_devices=8)

    xt_d = nc.dram_tensor("xt", [128, 4 * NDC * 512], BF16, kind="ExternalInput")
    wq_d = nc.dram_tensor("wq", [128, 2 * NDC * 256], BF16, kind="ExternalInput")
    wkv_d = nc.dram_tensor("wkv", [128, NDC * 256], BF16, kind="ExternalInput")
    wo_d = nc.dram_tensor("wo", [128, 2 * 2 * 2048], BF16, kind="ExternalInput")
    fm_d = nc.dram_tensor("fm", [64, S], F32, kind="ExternalInput")
    fp_d = nc.dram_tensor("fp", [64, S], F32, kind="ExternalInput")
    msk_d = nc.dram_tensor("msk", [128, 256], BF16, kind="ExternalInput")
    ones_d = nc.dram_tensor("ones", [128, 128], F32, kind="ExternalInput")
    onesb_d = nc.dram_tensor("onesb", [128, 128], BF16, kind="ExternalInput")
    out_d = nc.dram_tensor("out", [S, DIM], BF16, kind="ExternalOutput")

    with tile.TileContext(nc) as tc, ExitStack() as ctx:
        # ---- persistent pools (live across both phases) ----
        qp = ctx.enter_context(tc.tile_pool(name="qp", bufs=1))
        kp = ctx.enter_context(tc.tile_pool(name="kp", bufs=1))
        vp = ctx.enter_context(tc.tile_pool(name="vp", bufs=1))
        ap_ = ctx.enter_context(tc.tile_pool(name="ap", bufs=1))
        cp = ctx.enter_context(tc.tile_pool(name="cp", bufs=1))

        q1 = qp.tile([128, HPC, S], BF16, tag="q1")
        # rope-imag features, zero-padded to K=128 so score matmuls use a full
        # FWL-eligible stationary shared across the head pair
        q2p = qp.tile([128, HPC, S], BF16, tag="q2p")
        k1 = kp.tile([128, S], BF16, tag="k1")
        k2d = kp.tile([128, S], BF16, tag="k2d")  # k-imag, rows 64:128 zero
        v_sb = vp.tile([128, NQC, 128], BF16, tag="v")
        attn = ap_.tile([128, HPC, S], BF16, tag="attn")
        msk_t = cp.tile([128, 256], BF16, tag="msk")
        ones_t = cp.tile([128, 128], F32, tag="ones")
        onesb_t = cp.tile([128, 128], BF16, tag="onesb")

        nc.vector.memset(q2p[64:128, :, :], 0.0)
        nc.vector.memset(k2d[64:128, :], 0.0)

        # =================== phase 1: projections ===================
        with tc.tile_pool(name="xp", bufs=3) as xp, \
             tc.tile_pool(name="wp", bufs=1) as wp, \
             tc.tile_pool(name="rp", bufs=1) as rp, \
             tc.tile_pool(name="pps", bufs=5, space="PSUM") as pps:
            wkv_t = wp.tile([128, NDC, 256], BF16, tag="wkv")
            for i in range(2):
                nc.sync.dma_start(
                    wkv_t[:, i * 8 : (i + 1) * 8, :],
                    wkv_d[:, i * 8 * 256 : (i + 1) * 8 * 256],
                )
            wq_t = None
            fm = fp = None
            for cq in range(4):
                x_q = xp.tile([128, NDC, 512], BF16, tag="x")
                ndg = 8 if cq == 0 else 4
                w_dg = NDC // ndg
                for dg in range(ndg):
                    nc.sync.dma_start(
                        x_q[:, dg * w_dg : (dg + 1) * w_dg, :],
                        xt_d[
                            :,
                            cq * NDC * 512 + dg * w_dg * 512 : cq * NDC * 512
                            + (dg + 1) * w_dg * 512,
                        ],
                    )
                if cq == 0:
                    wq_t = [
                        wp.tile([128, NDC, 256], BF16, tag=f"wq{i}", name=f"wq{i}")
                        for i in range(2)
                    ]
                    for i in range(2):
                        nc.sync.dma_start(
                            wq_t[i][:], wq_d[:, i * NDC * 256 : (i + 1) * NDC * 256]
                        )
                    fm = rp.tile([64, S], F32, tag="fm")
                    nc.sync.dma_start(fm[:], fm_d[:, :])
                    fp = rp.tile([64, S], F32, tag="fp")
                    nc.sync.dma_start(fp[:], fp_d[:, :])
                    nc.sync.dma_start(msk_t[:], msk_d[:, :])
                    nc.sync.dma_start(ones_t[:], ones_d[:, :])
                    nc.sync.dma_start(onesb_t[:], onesb_d[:, :])
                cs = slice(cq * 512, (cq + 1) * 512)
                fmc, fpc = fm[:, cs], fp[:, cs]

                kps = pps.tile([128, 512], F32, tag="ps")
                for dc in range(NDC):
                    nc.tensor.matmul(
                        kps[:],
                        wkv_t[:, dc, 0:128],
                        x_q[:, dc, :],
                        start=(dc == 0),
                        stop=(dc == NDC - 1),
                    )
                nc.vector.tensor_mul(k1[0:64, cs], kps[0:64, :], fmc)
                nc.vector.tensor_mul(k1[64:128, cs], kps[0:64, :], fpc)
                nc.scalar.copy(k2d[0:64, cs], kps[64:128, :])

                vps = pps.tile([128, 512], F32, tag="ps")
                for kb4 in range(4):
                    kb = cq * 4 + kb4
                    for dc in range(NDC):
                        nc.tensor.matmul(
                            vps[:, kb4 * 128 : (kb4 + 1) * 128],
                            x_q[:, dc, kb4 * 128 : (kb4 + 1) * 128],
                            wkv_t[:, dc, 128:256],
                            start=(dc == 0),
                            stop=(dc == NDC - 1),
                        )
                for kb4 in range(4):
                    nc.any.tensor_copy(
                        v_sb[:, cq * 4 + kb4, :], vps[:, kb4 * 128 : (kb4 + 1) * 128]
                    )

                for h in range(HPC):
                    qps = pps.tile([128, 512], F32, tag="ps")
                    for dc in range(NDC):
                        nc.tensor.matmul(
                            qps[:],
                            wq_t[h // 2][:, dc, (h % 2) * 128 : (h % 2 + 1) * 128],
                            x_q[:, dc, :],
                            start=(dc == 0),
                            stop=(dc == NDC - 1),
                        )
                    nc.vector.tensor_mul(q1[0:64, h, cs], qps[0:64, :], fmc)
                    nc.vector.tensor_mul(q1[64:128, h, cs], qps[0:64, :], fpc)
                    nc.scalar.copy(q2p[0:64, h, cs], qps[64:128, :])

        # =================== phase 2: attention + O-proj ===================
        with tc.tile_pool(name="wop", bufs=1) as wop, \
             tc.tile_pool(name="ptp", bufs=10) as ptp, \
             tc.tile_pool(name="rip", bufs=2) as rip, \
             tc.tile_pool(name="obp", bufs=2) as obp, \
             tc.tile_pool(name="sps", bufs=5, space="PSUM") as sps, \
             tc.tile_pool(name="aop", bufs=3, space="PSUM") as aop:
            wo_t = [
                wop.tile([128, 2, 2048], BF16, tag=f"wo{i}", name=f"wo{i}")
                for i in range(2)
            ]
            for i in range(2):
                nc.sync.dma_start(wo_t[i][:], wo_d[:, i * 4096 : (i + 1) * 4096])

            ptiles = {}

            def scores(kb):
                kwl = kb * 128
                nw = min(kb + 8, NQC - 1) - kb + 1
                w = nw * 128
                ptk = ptp.tile([128, HPC, 1152], BF16, tag="pt", name=f"pt{kb}")
                ptiles[kb] = ptk
                for p in range(2):
                    h0, h1 = 2 * p, 2 * p + 1
                    for c0 in range(0, w, 512):
                        c1 = min(w, c0 + 512)
                        cw = c1 - c0
                        sp0 = sps.tile([128, 512], F32, tag="s", name="sp0")
                        sp1 = sps.tile([128, 512], F32, tag="s", name="sp1")
                        nc.tensor.matmul(
                            sp0[:, 0:cw], k1[:, kwl : kwl + 128],
                            q1[:, h0, kwl + c0 : kwl + c1], start=True, stop=False,
                        )
                        nc.tensor.matmul(
                            sp1[:, 0:cw], k1[:, kwl : kwl + 128],
                            q1[:, h1, kwl + c0 : kwl + c1], start=True, stop=False,
                        )
                        nc.tensor.matmul(
                            sp0[:, 0:cw], k2d[:, kwl : kwl + 128],
                            q2p[:, h0, kwl + c0 : kwl + c1], start=False, stop=True,
                        )
                        nc.tensor.matmul(
                            sp1[:, 0:cw], k2d[:, kwl : kwl + 128],
                            q2p[:, h1, kwl + c0 : kwl + c1], start=False, stop=True,
                        )
                        nc.scalar.activation(ptk[:, h0, c0:c1], sp0[:, 0:cw], EXP)
                        nc.scalar.activation(ptk[:, h1, c0:c1], sp1[:, 0:cw], EXP)
            def masks(kb):
                # causal triangle on the diagonal block, window triangle on the tail
                nw = min(kb + 8, NQC - 1) - kb + 1
                ptk = ptiles[kb]
                for h in range(HPC):
                    nc.gpsimd.tensor_mul(
                        ptk[:, h, 0:128], ptk[:, h, 0:128], msk_t[:, 0:128]
                    )
                    if nw == 9:
                        nc.gpsimd.tensor_mul(
                            ptk[:, h, 1024:1152], ptk[:, h, 1024:1152],
                            msk_t[:, 128:256],
                        )

            def finish(qc):
                klo = max(0, qc - 8)
                kbs = list(range(klo, qc + 1))
                # rowsums: col-tiled ones-matmuls — the 4 heads run concurrently
                # in distinct 32-column strips of the PE array
                rsum = aop.tile([128, HPC, 128], F32, tag="x", name="rsum")
                for i, kb2 in enumerate(kbs):
                    qoff = (qc - kb2) * 128
                    nc.tensor.matmul(
                        rsum[:],
                        onesb_t[:, 0:128],
                        ptiles[kb2][:, :, qoff : qoff + 128],
                        start=(i == 0),
                        stop=(i == len(kbs) - 1),
                    )
                bcs = rip.tile([128, HPC, 128], F32, tag="bcs")
                nc.vector.reciprocal_approx_fast(bcs[:], rsum[:])
                av = aop.tile([128, HPC, 128], F32, tag="x", name="av")
                for i, kb2 in enumerate(kbs):
                    qoff = (qc - kb2) * 128
                    nc.tensor.matmul(
                        av[:],
                        v_sb[:, kb2, :],
                        ptiles[kb2][:, :, qoff : qoff + 128],
                        start=(i == 0),
                        stop=(i == len(kbs) - 1),
                    )
                nc.vector.tensor_mul(
                    attn[:, :, qc * 128 : (qc + 1) * 128], av[:], bcs[:]
                )

            def oproj(qc):
                for dn in range(4):
                    op = aop.tile([128, 512], F32, tag="x", name="op")
                    for f in range(HPC):
                        nc.tensor.matmul(
                            op[:],
                            attn[:, f, qc * 128 : (qc + 1) * 128],
                            wo_t[f // 2][:, f % 2, dn * 512 : (dn + 1) * 512],
                            start=(f == 0),
                            stop=(f == HPC - 1),
                        )
                    osb = obp.tile([128, 512], BF16, tag="ob")
                    nc.vector.tensor_copy(osb[:], op[:])
                    nc.sync.dma_start(
                        out_d[qc * 128 : (qc + 1) * 128, dn * 512 : (dn + 1) * 512],
                        osb[:],
                    )

            for s in range(NQC + 2):
                if s < NQC:
                    scores(s)
                if s < NQC:
                    masks(s)
                if 1 <= s <= NQC:
                    finish(s - 1)
                if s >= 2:
                    oproj(s - 2)

    nc.compile()
    return nc


def _prep_core(inputs, c):
    x = inputs["x"]
    cos, sin = np.asarray(inputs["cos"]), np.asarray(inputs["sin"])
    wq = np.asarray(inputs["wq"], dtype=np.float32)
    wk = np.asarray(inputs["wk"], dtype=np.float32)
    wv = np.asarray(inputs["wv"], dtype=np.float32)
    wo = np.asarray(inputs["wo"], dtype=np.float32)
    bf = ml_dtypes.bfloat16
    b, g = c // 4, c % 4

    # x[b] transposed -> [128p, cq, dc, 512]
    xt = np.asarray(x[b], dtype=np.float32).T  # [dim, S]
    xt = xt.reshape(NDC, 128, 4, 512).transpose(1, 2, 0, 3)
    xt = np.ascontiguousarray(xt).reshape(128, 4 * NDC * 512).astype(bf)

    # wq slice for heads 4g..4g+3 (SCALE folded), [p, hpair, dc, 256]
    wqs = (wq[:, g * 512 : (g + 1) * 512] * SCALE).reshape(NDC, 128, 2, 256)
    wqs = np.ascontiguousarray(wqs.transpose(1, 2, 0, 3)).reshape(128, 2 * NDC * 256)
    # wk|wv slice for kv head g: [p, dc, 256] with cols [wk 128 | wv 128]
    wkv = np.concatenate(
        [wk[:, g * 128 : (g + 1) * 128], wv[:, g * 128 : (g + 1) * 128]], axis=1
    )
    wkv = np.ascontiguousarray(wkv.reshape(NDC, 128, 256).transpose(1, 0, 2)).reshape(
        128, NDC * 256
    )
    # wo rows for this core's heads: [p, pair, head-in-pair, dim]
    wos = wo[g * 512 : (g + 1) * 512].reshape(2, 2, 128, 2048).transpose(2, 0, 1, 3)
    wos = np.ascontiguousarray(wos).reshape(128, 2 * 2 * 2048)

    fm = np.ascontiguousarray((cos - sin).T, dtype=np.float32)
    fp_ = np.ascontiguousarray((cos + sin).T, dtype=np.float32)

    ki = np.arange(128)[:, None]
    qi = np.arange(128)[None, :]
    msk = np.concatenate(
        [(ki <= qi).astype(np.float32), (ki > qi).astype(np.float32)], axis=1
    )

    return {
        "xt": xt, "wq": wqs.astype(bf), "wkv": wkv.astype(bf), "wo": wos.astype(bf),
        "fm": fm, "fp": fp_,
        "msk": msk.astype(bf), "ones": np.ones((128, 128), dtype=np.float32),
        "onesb": np.ones((128, 128), dtype=np.float32).astype(bf),
    }


def kernel(**inputs) -> np.ndarray:
    if "nc" not in _cache:
        _cache["nc"] = _build()
    nc = _cache["nc"]
    in_maps = [_prep_core(inputs, c) for c in range(8)]
    res = run_bass_kernel_spmd(nc, in_maps, core_ids=list(range(8)))
    out = np.zeros((B, S, DIM), dtype=np.float32)
    for c in range(8):
        out[c // 4] += res.results[c]["out"].astype(np.float32)
    return out


# revision 24
# speedup vs baseline: 1.1788x; 1.1788x over previous
"""Distributed Bass kernel for sliding-window GQA attention on 8 TRN2 NeuronCores.

Problem: B=2, S=2048, DIM=2048, H=16, KVH=4, HD=128, WINDOW=1024 (causal
sliding window), nonstandard RoPE producing 1.5*HD score features.

Sharding (tensor-parallel on the kv-head axis, data-parallel on batch —
no collectives): core c owns (batch, kv-group) = (c//4, c%4): its 4 q-heads
and 1 kv head over the full 2048-row sequence. wq/wk/wv are column-sharded
by kv group, wo row-sharded. Each core emits a PARTIAL output projection
(its 4 heads x its wo rows); the host sums the 4 partials per batch while
unsharding — replacing the all-reduce.

v2 layout: scores are computed TRANSPOSED (S^T[k, q], K-block stationary and
shared across the 4 heads; the 64-wide rope-imag contraction is row-tiled so
head pairs run concurrently in the PE array). exp goes straight to SBUF as
P^T — no per-block transpose matmuls. Causal/window triangles are applied as
0/1 multiplies on the otherwise-idle GpSimd engine after exp. Row sums come
from a ones-stationary matmul (output replicated across partitions, packed
(h, q)); PV packs all 4 heads into N=512 matmuls against the shared V; the
1/rowsum normalization is fused into the PSUM->SBUF evacuation of PV.
"""
import numpy as np
import ml_dtypes

import concourse.tile as tile
from concourse import bacc, mybir
from concourse.bass_utils import run_bass_kernel_spmd
from contextlib import ExitStack

F32 = mybir.dt.float32
BF16 = mybir.dt.bfloat16
EXP = mybir.ActivationFunctionType.Exp

B, S, DIM = 2, 2048, 2048
H, KVH, HD = 16, 4, 128
HPC = H // KVH  # heads per core (4)
WINDOW = 1024
SCALE = HD ** -0.5
NDC = DIM // 128  # 16 dim chunks
NQC = S // 128    # 16 q/k blocks

_cache = {}


def _build():
    nc = bacc.Bacc("TRN2", target_bir_lowering=False, debug=False, num_devices=8)

    xt_d = nc.dram_tensor("xt", [128, 4 * NDC * 512], BF16, kind="ExternalInput")
    wq_d = nc.dram_tensor("wq", [128, 2 * NDC * 256], BF16, kind="ExternalInput")
    wkv_d = nc.dram_tensor("wkv", [128, NDC * 256], BF16, kind="ExternalInput")
    wo_d = nc.dram_tensor("wo", [128, 2 * 2 * 2048], BF16, kind="ExternalInput")
    fm_d = nc.dram_tensor("fm", [64, S], F32, kind="ExternalInput")
    fp_d = nc.dram_tensor("fp", [64, S], F32, kind="ExternalInput")
    msk_d = nc.dram_tensor("msk", [128, 256], BF16, kind="ExternalInput")
    ones_d = nc.dram_tensor("ones", [128, 128], F32, kind="ExternalInput")
    onesb_d = nc.dram_tensor("onesb", [128, 128], BF16, kind="ExternalInput")
    out_d = nc.dram_tensor("out", [S, DIM], BF16, kind="ExternalOutput")

    with tile.TileContext(nc) as tc, ExitStack() as ctx:
        # ---- persistent pools (live across both phases) ----
        qp = ctx.enter_context(tc.tile_pool(name="qp", bufs=1))
        kp = ctx.enter_context(tc.tile_pool(name="kp", bufs=1))
        vp = ctx.enter_context(tc.tile_pool(name="vp", bufs=1))
        ap_ = ctx.enter_context(tc.tile_pool(name="ap", bufs=1))
        cp = ctx.enter_context(tc.tile_pool(name="cp", bufs=1))

        q1 = qp.tile([128, HPC, S], BF16, tag="q1")
        # rope-imag features, zero-padded to K=128 so score matmuls use a full
        # FWL-eligible stationary shared across the head pair
        q2p = qp.tile([128, HPC, S], BF16, tag="q2p")
        k1 = kp.tile([128, S], BF16, tag="k1")
        k2d = kp.tile([128, S], BF16, tag="k2d")  # k-imag, rows 64:128 zero
        v_sb = vp.tile([128, NQC, 128], BF16, tag="v")
        attn = ap_.tile([128, HPC, S], BF16, tag="attn")
        msk_t = cp.tile([128, 256], BF16, tag="msk")
        ones_t = cp.tile([128, 128], F32, tag="ones")
        onesb_t = cp.tile([128, 128], BF16, tag="onesb")

        nc.vector.memset(q2p[64:128, :, :], 0.0)
        nc.vector.memset(k2d[64:128, :], 0.0)

        # =================== phase 1: projections ===================
        with tc.tile_pool(name="xp", bufs=3) as xp, \
             tc.tile_pool(name="wp", bufs=1) as wp, \
             tc.tile_pool(name="rp", bufs=1) as rp, \
             tc.tile_pool(name="pps", bufs=5, space="PSUM") as pps:
            wkv_t = wp.tile([128, NDC, 256], BF16, tag="wkv")
            nc.sync.dma_start(wkv_t[:, 0:2, :], wkv_d[:, 0:512])
            nc.sync.dma_start(wkv_t[:, 2:9, :], wkv_d[:, 512 : 9 * 256])
            nc.sync.dma_start(wkv_t[:, 9:16, :], wkv_d[:, 9 * 256 : 16 * 256])
            wq_t = None
            fm = fp = None
            for cq in range(4):
                x_q = xp.tile([128, NDC, 512], BF16, tag="x")
                ndg = 8 if cq == 0 else 4
                w_dg = NDC // ndg
                for dg in range(ndg):
                    nc.sync.dma_start(
                        x_q[:, dg * w_dg : (dg + 1) * w_dg, :],
                        xt_d[
                            :,
                            cq * NDC * 512 + dg * w_dg * 512 : cq * NDC * 512
                            + (dg + 1) * w_dg * 512,
                        ],
                    )
                if cq == 0:
                    wq_t = [
                        wp.tile([128, NDC, 256], BF16, tag=f"wq{i}", name=f"wq{i}")
                        for i in range(2)
                    ]
                    for i in range(2):
                        nc.sync.dma_start(
                            wq_t[i][:], wq_d[:, i * NDC * 256 : (i + 1) * NDC * 256]
                        )
                    fm = rp.tile([64, S], F32, tag="fm")
                    nc.sync.dma_start(fm[:], fm_d[:, :])
                    fp = rp.tile([64, S], F32, tag="fp")
                    nc.sync.dma_start(fp[:], fp_d[:, :])
                    nc.sync.dma_start(msk_t[:], msk_d[:, :])
                    nc.sync.dma_start(ones_t[:], ones_d[:, :])
                    nc.sync.dma_start(onesb_t[:], onesb_d[:, :])
                cs = slice(cq * 512, (cq + 1) * 512)
                fmc, fpc = fm[:, cs], fp[:, cs]

                kps = pps.tile([128, 512], F32, tag="ps")
                for dc in range(NDC):
                    nc.tensor.matmul(
                        kps[:],
                        wkv_t[:, dc, 0:128],
                        x_q[:, dc, :],
                        start=(dc == 0),
                        stop=(dc == NDC - 1),
                    )
                nc.vector.tensor_mul(k1[0:64, cs], kps[0:64, :], fmc)
                nc.vector.tensor_mul(k1[64:128, cs], kps[0:64, :], fpc)
                nc.scalar.copy(k2d[0:64, cs], kps[64:128, :])

                vps = pps.tile([128, 512], F32, tag="ps")
                for kb4 in range(4):
                    kb = cq * 4 + kb4
                    for dc in range(NDC):
                        nc.tensor.matmul(
                            vps[:, kb4 * 128 : (kb4 + 1) * 128],
                            x_q[:, dc, kb4 * 128 : (kb4 + 1) * 128],
                            wkv_t[:, dc, 128:256],
                            start=(dc == 0),
                            stop=(dc == NDC - 1),
                        )
                for kb4 in range(4):
                    nc.any.tensor_copy(
                        v_sb[:, cq * 4 + kb4, :], vps[:, kb4 * 128 : (kb4 + 1) * 128]
                    )

                for h in range(HPC):
                    qps = pps.tile([128, 512], F32, tag="ps")
                    for dc in range(NDC):
                        nc.tensor.matmul(
                            qps[:],
                            wq_t[h // 2][:, dc, (h % 2) * 128 : (h % 2 + 1) * 128],
                            x_q[:, dc, :],
                            start=(dc == 0),
                            stop=(dc == NDC - 1),
                        )
                    nc.vector.tensor_mul(q1[0:64, h, cs], qps[0:64, :], fmc)
                    nc.vector.tensor_mul(q1[64:128, h, cs], qps[0:64, :], fpc)
                    nc.scalar.copy(q2p[0:64, h, cs], qps[64:128, :])

        # =================== phase 2: attention + O-proj ===================
        with tc.tile_pool(name="wop", bufs=1) as wop, \
             tc.tile_pool(name="ptp", bufs=10) as ptp, \
             tc.tile_pool(name="rip", bufs=2) as rip, \
             tc.tile_pool(name="obp", bufs=2) as obp, \
             tc.tile_pool(name="sps", bufs=4, space="PSUM") as sps, \
             tc.tile_pool(name="rsp", bufs=1, space="PSUM") as rsp, \
             tc.tile_pool(name="avp", bufs=1, space="PSUM") as avp, \
             tc.tile_pool(name="ops", bufs=2, space="PSUM") as ops:
            wo_t = [
                wop.tile([128, 2, 2048], BF16, tag=f"wo{i}", name=f"wo{i}")
                for i in range(2)
            ]
            for i in range(2):
                nc.sync.dma_start(wo_t[i][:], wo_d[:, i * 4096 : (i + 1) * 4096])

            ptiles = {}

            def scores(kb):
                kwl = kb * 128
                nw = min(kb + 8, NQC - 1) - kb + 1
                w = nw * 128
                ptk = ptp.tile([128, HPC, 1152], BF16, tag="pt", name=f"pt{kb}")
                ptiles[kb] = ptk
                for p in range(2):
                    h0, h1 = 2 * p, 2 * p + 1
                    for c0 in range(0, w, 512):
                        c1 = min(w, c0 + 512)
                        cw = c1 - c0
                        sp0 = sps.tile([128, 512], F32, tag="s", name="sp0")
                        sp1 = sps.tile([128, 512], F32, tag="s", name="sp1")
                        nc.tensor.matmul(
                            sp0[:, 0:cw], k1[:, kwl : kwl + 128],
                            q1[:, h0, kwl + c0 : kwl + c1], start=True, stop=False,
                        )
                        nc.tensor.matmul(
                            sp1[:, 0:cw], k1[:, kwl : kwl + 128],
                            q1[:, h1, kwl + c0 : kwl + c1], start=True, stop=False,
                        )
                        nc.tensor.matmul(
                            sp0[:, 0:cw], k2d[:, kwl : kwl + 128],
                            q2p[:, h0, kwl + c0 : kwl + c1], start=False, stop=True,
                        )
                        nc.tensor.matmul(
                            sp1[:, 0:cw], k2d[:, kwl : kwl + 128],
                            q2p[:, h1, kwl + c0 : kwl + c1], start=False, stop=True,
                        )
                        nc.scalar.activation(ptk[:, h0, c0:c1], sp0[:, 0:cw], EXP)
                        nc.scalar.activation(ptk[:, h1, c0:c1], sp1[:, 0:cw], EXP)
            def masks(kb):
                # causal triangle on the diagonal block, window triangle on the tail
                nw = min(kb + 8, NQC - 1) - kb + 1
                ptk = ptiles[kb]
                for h in range(HPC):
                    nc.gpsimd.tensor_mul(
                        ptk[:, h, 0:128], ptk[:, h, 0:128], msk_t[:, 0:128]
                    )
                    if nw == 9:
                        nc.gpsimd.tensor_mul(
                            ptk[:, h, 1024:1152], ptk[:, h, 1024:1152],
                            msk_t[:, 128:256],
                        )

            def finish(qc):
                klo = max(0, qc - 8)
                kbs = list(range(klo, qc + 1))
                # rowsums: col-tiled ones-matmuls — the 4 heads run concurrently
                # in distinct 32-column strips of the PE array
                rsum = rsp.tile([128, HPC, 128], F32, tag="rsbc", name="rsum")
                for i, kb2 in enumerate(kbs):
                    qoff = (qc - kb2) * 128
                    nc.tensor.matmul(
                        rsum[:],
                        onesb_t[:, 0:128],
                        ptiles[kb2][:, :, qoff : qoff + 128],
                        start=(i == 0),
                        stop=(i == len(kbs) - 1),
                    )
                bcs = rip.tile([128, HPC, 128], F32, tag="bcs")
                nc.vector.reciprocal_approx_fast(bcs[:], rsum[:])
                av = avp.tile([128, HPC, 128], F32, tag="av")
                for i, kb2 in enumerate(kbs):
                    qoff = (qc - kb2) * 128
                    nc.tensor.matmul(
                        av[:],
                        v_sb[:, kb2, :],
                        ptiles[kb2][:, :, qoff : qoff + 128],
                        start=(i == 0),
                        stop=(i == len(kbs) - 1),
                    )
                nc.vector.tensor_mul(
                    attn[:, :, qc * 128 : (qc + 1) * 128], av[:], bcs[:]
                )

            def oproj(qc):
                for dn in range(4):
                    op = ops.tile([128, 512], F32, tag="o")
                    for f in range(HPC):
                        nc.tensor.matmul(
                            op[:],
                            attn[:, f, qc * 128 : (qc + 1) * 128],
                            wo_t[f // 2][:, f % 2, dn * 512 : (dn + 1) * 512],
                            start=(f == 0),
                            stop=(f == HPC - 1),
                        )
                    osb = obp.tile([128, 512], BF16, tag="ob")
                    nc.vector.tensor_copy(osb[:], op[:])
                    nc.sync.dma_start(
                        out_d[qc * 128 : (qc + 1) * 128, dn * 512 : (dn + 1) * 512],
                        osb[:],
                    )

            for s in range(NQC + 2):
                if s < NQC:
                    scores(s)
                if s < NQC:
                    masks(s)
                if 1 <= s <= NQC:
                    finish(s - 1)
                if s >= 2:
                    oproj(s - 2)

    nc.compile()
    return nc


def _prep_core(inputs, c):
    x = inputs["x"]
    cos, sin = np.asarray(inputs["cos"]), np.asarray(inputs["sin"])
    wq = np.asarray(inputs["wq"], dtype=np.float32)
    wk = np.asarray(inputs["wk"], dtype=np.float32)
    wv = np.asarray(inputs["wv"], dtype=np.float32)
    wo = np.asarray(inputs["wo"], dtype=np.float32)
    bf = ml_dtypes.bfloat16
    b, g = c // 4, c % 4

    # x[b] transposed -> [128p, cq, dc, 512]
    xt = np.asarray(x[b], dtype=np.float32).T  # [dim, S]
    xt = xt.reshape(NDC, 128, 4, 512).transpose(1, 2, 0, 3)
    xt = np.ascontiguousarray(xt).reshape(128, 4 * NDC * 512).astype(bf)

    # wq slice for heads 4g..4g+3 (SCALE folded), [p, hpair, dc, 256]
    wqs = (wq[:, g * 512 : (g + 1) * 512] * SCALE).reshape(NDC, 128, 2, 256)
    wqs = np.ascontiguousarray(wqs.transpose(1, 2, 0, 3)).reshape(128, 2 * NDC * 256)
    # wk|wv slice for kv head g: [p, dc, 256] with cols [wk 128 | wv 128]
    wkv = np.concatenate(
        [wk[:, g * 128 : (g + 1) * 128], wv[:, g * 128 : (g + 1) * 128]], axis=1
    )
    wkv = np.ascontiguousarray(wkv.reshape(NDC, 128, 256).transpose(1, 0, 2)).reshape(
        128, NDC * 256
    )
    # wo rows for this core's heads: [p, pair, head-in-pair, dim]
    wos = wo[g * 512 : (g + 1) * 512].reshape(2, 2, 128, 2048).transpose(2, 0, 1, 3)
    wos = np.ascontiguousarray(wos).reshape(128, 2 * 2 * 2048)

    fm = np.ascontiguousarray((cos - sin).T, dtype=np.float32)
    fp_ = np.ascontiguousarray((cos + sin).T, dtype=np.float32)

    ki = np.arange(128)[:, None]
    qi = np.arange(128)[None, :]
    msk = np.concatenate(
        [(ki <= qi).astype(np.float32), (ki > qi).astype(np.float32)], axis=1
    )

    return {
        "xt": xt, "wq": wqs.astype(bf), "wkv": wkv.astype(bf), "wo": wos.astype(bf),
        "fm": fm, "fp": fp_,
        "msk": msk.astype(bf), "ones": np.ones((128, 128), dtype=np.float32),
        "onesb": np.ones((128, 128), dtype=np.float32).astype(bf),
    }


def kernel(**inputs) -> np.ndarray:
    if "nc" not in _cache:
        _cache["nc"] = _build()
    nc = _cache["nc"]
    in_maps = [_prep_core(inputs, c) for c in range(8)]
    res = run_bass_kernel_spmd(nc, in_maps, core_ids=list(range(8)))
    out = np.zeros((B, S, DIM), dtype=np.float32)
    for c in range(8):
        out[c // 4] += res.results[c]["out"].astype(np.float32)
    return out
